# revision 1
# baseline (speedup 1.0000x reference)
"""Trainium2 Bass kernel for nn_MultiHeadAttention (softmax over HEAD axis).

Problem: B=2, T=2048, D=1024, H=16, HD=64.
  Q,K,V = x@W* + b*;  score = QK^T/32 with causal positions set to -1e10
  weight = softmax(score, axis=HEADS)  -> masked (j>i) entries get exactly 1/16
  out = weight@V;  y = out@Wo + bo

Exact identity used: for row i,
  out_h[i] = sum_{j<=i} w_h[i,j] V_h[j] + (1/16) sum_{j>i} V_h[j]
where w is the head-softmax of unmasked scores. We compute softmax weights
only on causal j-blocks, zero the off-causal entries via 0/1 masks, and add
the (1/16)*suffix-sum(V) correction as a host-precomputed matrix (V comes
from launch A's own output, so the correction is consistent to fp16).

Sharding (8 cores, two launches):
  Launch A: QKV projections, 8-way token-sharded.
  Launch B: attention + out-proj. Core c (q = c%4, batch c//4) handles the
    mirrored 2-block chunks A=(2q, 2q+1), B=(14-2q, 15-2q) of 128-row blocks.
    One SPMD program for all cores: slot A runs 8 j-block positions, slot B
    16; real causal counts are (2q+2, 16-2q) and the rest are padding whose
    weights the per-core mask data zeroes. Total exact work is equal on all
    cores (18 positions); padding adds 6.

All matmul inputs fp16 (1 cyc/row on PE), accumulation fp32 in PSUM.
"""

import numpy as np

import concourse.bass as bass
import concourse.tile as tile
from concourse import bacc, mybir
from concourse.bass_utils import run_bass_kernel_spmd

F16 = mybir.dt.float16
F32 = mybir.dt.float32
AF = mybir.ActivationFunctionType

B, T, D, H, HD = 2, 2048, 1024, 16, 64
NC = 8
NBLK = T // 128          # 16
CNT = (8, 16)            # padded j-position counts for slot A / slot B
NPOS = CNT[0] + CNT[1]   # 24

# head slot order per 4-head score group: even (row-group-0) heads first so
# each PSUM bank only ever receives matmuls from one PE row group.
_GRP_HEADS = [[4 * g, 4 * g + 2, 4 * g + 1, 4 * g + 3] for g in range(4)]
# head -> (group, slot)
_HEAD_SLOT = {}
for _g in range(4):
    for _s, _h in enumerate(_GRP_HEADS[_g]):
        _HEAD_SLOT[_h] = (_g, _s)

_cache: dict = {}


# ----------------------------------------------------------------- launch A
def _build_a(reps=1):
    """QKV projections for a 512-token slice (8-way token-sharded)."""
    nc = bacc.Bacc("TRN2", target_bir_lowering=False, debug=False, num_devices=NC)
    xT = nc.dram_tensor("xT", [128, 8, 512], F16, kind="ExternalInput")
    wq = nc.dram_tensor("wq", [128, 8, D], F16, kind="ExternalInput")
    wk = nc.dram_tensor("wk", [128, 8, D], F16, kind="ExternalInput")
    wv = nc.dram_tensor("wv", [128, 8, D], F16, kind="ExternalInput")
    bqT = nc.dram_tensor("bqT", [128, 8], F32, kind="ExternalInput")
    bkT = nc.dram_tensor("bkT", [128, 8], F32, kind="ExternalInput")
    bv_row = nc.dram_tensor("bv_row", [1, D], F16, kind="ExternalInput")
    qT_o = nc.dram_tensor("qT_o", [128, 8, 512], F16, kind="ExternalOutput")
    kT_o = nc.dram_tensor("kT_o", [128, 8, 512], F16, kind="ExternalOutput")
    v_o = nc.dram_tensor("v_o", [128, 4, D], F16, kind="ExternalOutput")

    from contextlib import nullcontext
    with tile.TileContext(nc) as tc:
        with (tc.For_i(0, reps) if reps > 1 else nullcontext()), \
             tc.tile_pool(name="sg", bufs=1) as sg, \
             tc.tile_pool(name="out", bufs=1) as outp, \
             tc.tile_pool(name="ps", bufs=8, space="PSUM") as ps:
            xt = sg.tile([128, 8, 512], F16, tag="xt")
            nc.sync.dma_start(out=xt[:], in_=xT[:])
            wts = {}
            for nm, dram in (("wq", wq), ("wk", wk), ("wv", wv)):
                wt = sg.tile([128, 8, D], F16, tag=nm)
                nc.sync.dma_start(out=wt[:], in_=dram[:])
                wts[nm] = wt
            bq_sb = sg.tile([128, 8], F32, tag="bq")
            nc.sync.dma_start(out=bq_sb[:], in_=bqT[:])
            bk_sb = sg.tile([128, 8], F32, tag="bk")
            nc.sync.dma_start(out=bk_sb[:], in_=bkT[:])
            bv_sb = sg.tile([1, D], F16, tag="bv")
            nc.sync.dma_start(out=bv_sb[:], in_=bv_row[:])
            ones1 = sg.tile([1, 128], F16, tag="ones1")
            nc.vector.memset(ones1[:], 1.0)

            # Q^T, K^T: out[dout_chunk, t] = W[din, dout].T @ xT[din, t]
            for nm, bias_sb, scale, dst in (
                ("wq", bq_sb, 1.0, qT_o),
                ("wk", bk_sb, 1.0 / 32.0, kT_o),
            ):
                res = outp.tile([128, 8, 512], F16, tag=f"r{nm}")
                for m in range(8):
                    acc = ps.tile([128, 512], F32, tag="acc")
                    for k in range(8):
                        nc.tensor.matmul(
                            acc[:],
                            wts[nm][:, k, m * 128:(m + 1) * 128],
                            xt[:, k, :],
                            start=(k == 0), stop=(k == 7),
                        )
                    nc.scalar.activation(
                        out=res[:, m, :], in_=acc[:], func=AF.Identity,
                        bias=bias_sb[:, m:m + 1], scale=scale,
                    )
                nc.sync.dma_start(out=dst[:], in_=res[:])

            # V natural: out[t_chunk, dout] = xT[din, t_chunk].T @ Wv[din, dout]
            rv = outp.tile([128, 4, D], F16, tag="rv")
            for tcn in range(4):
                for nt in range(2):
                    acc = ps.tile([128, 512], F32, tag="acc")
                    for k in range(8):
                        nc.tensor.matmul(
                            acc[:],
                            xt[:, k, tcn * 128:(tcn + 1) * 128],
                            wts["wv"][:, k, nt * 512:(nt + 1) * 512],
                            start=(k == 0), stop=False,
                        )
                    nc.tensor.matmul(
                        acc[:], ones1[:], bv_sb[:, nt * 512:(nt + 1) * 512],
                        start=False, stop=True,
                    )
                    nc.scalar.activation(
                        out=rv[:, tcn, nt * 512:(nt + 1) * 512], in_=acc[:],
                        func=AF.Copy)
            nc.sync.dma_start(out=v_o[:], in_=rv[:])
    nc.compile()
    return nc


# ----------------------------------------------------------------- launch B
def _chunk_blocks(q):
    """Global 128-row block indices of the two chunks handled by quarter q."""
    return (2 * q, 2 * q + 1), (14 - 2 * q, 15 - 2 * q)


def _build_b(reps=1, stages=5, zdve=True):
    """Uniform attention program (same for all cores).

    Per-core inputs:
      qT [1024, 512] f16 : Q^T, cols = [chunk A 256 | chunk B 256]
      kT [1024, 2048] f16 (pre-scaled 1/32), v [2048, 1024] f16
      wo [1024, 1024] f16, bo_row [1, 1024] f16, ident [128, 128] f16
      masks [24, 128, 256] f16 : per position 0/1 weight-keep masks
      corr [2, 8, 128, 256] f16 : (1/16)*suffix-sum-of-V correction, as
          [chunk, d-pair-chunk, d-within, i-col] added to out^T
    Output: y [512, 1024] f32 (rows = [chunk A | chunk B]).
    """
    nc = bacc.Bacc("TRN2", target_bir_lowering=False, debug=False, num_devices=NC)
    qT = nc.dram_tensor("qT", [128, 8, 512], F16, kind="ExternalInput")
    kT = nc.dram_tensor("kT", [128, 8, T], F16, kind="ExternalInput")
    v = nc.dram_tensor("v", [128, 16, D], F16, kind="ExternalInput")
    wo = nc.dram_tensor("wo", [128, 8, D], F16, kind="ExternalInput")
    bo_row = nc.dram_tensor("bo_row", [1, D], F16, kind="ExternalInput")
    ident = nc.dram_tensor("ident", [128, 128], F16, kind="ExternalInput")
    masks = nc.dram_tensor("masks", [NPOS, 128, 256], F16, kind="ExternalInput")
    corr = nc.dram_tensor("corr", [2, 8, 128, 256], F16, kind="ExternalInput")
    y_o = nc.dram_tensor("y", [512, D], F32, kind="ExternalOutput")

    from contextlib import nullcontext
    with tile.TileContext(nc) as tc:
        with (tc.For_i(0, reps) if reps > 1 else nullcontext()), \
             tc.tile_pool(name="sg", bufs=1) as sg, \
             tc.tile_pool(name="wbuf", bufs=5) as wbuf, \
             tc.tile_pool(name="pt", bufs=5) as ptp, \
             tc.tile_pool(name="rt", bufs=3) as rtp, \
             tc.tile_pool(name="mk", bufs=8) as mkp, \
             tc.tile_pool(name="op", bufs=1) as opp, \
             tc.tile_pool(name="ysb", bufs=2) as ysbp:

            kt = sg.tile([128, 8, T], F16, tag="kt")
            nc.sync.dma_start(out=kt[:], in_=kT[:])
            qt = sg.tile([128, 8, 512], F16, tag="qt")
            nc.sync.dma_start(out=qt[:], in_=qT[:])
            vt = sg.tile([128, 16, D], F16, tag="vt")
            nc.sync.dma_start(out=vt[:], in_=v[:])
            wot = sg.tile([128, 8, D], F16, tag="wot")
            nc.sync.dma_start(out=wot[:], in_=wo[:])
            idt = sg.tile([128, 128], F16, tag="idt")
            nc.sync.dma_start(out=idt[:], in_=ident[:])
            bo_sb = sg.tile([1, D], F16, tag="bo")
            nc.sync.dma_start(out=bo_sb[:], in_=bo_row[:])
            ones1 = sg.tile([1, 128], F16, tag="ones1")
            nc.vector.memset(ones1[:], 1.0)

            # out^T partials per chunk: [128, nsb, 8 pairs, 256]
            outp_tiles = []

            with tc.tile_pool(name="score", bufs=2, space="PSUM") as scp, \
                 tc.tile_pool(name="z", bufs=1 if zdve else 2, space="PSUM") as zp, \
                 tc.tile_pool(name="ot", bufs=2, space="PSUM") as otp:
                for ci in range(2):
                    npos = CNT[ci]
                    coff = ci * 256
                    poff = 0 if ci == 0 else CNT[0]   # mask index offset
                    nsb = npos // 4                   # super-blocks of 2 pairs
                    outp_c = opp.tile([128, nsb, 8, 256], F16, tag=f"outp{ci}")
                    outp_tiles.append((outp_c, nsb))

                    for s in range(nsb):
                        wts_s = {}
                        # ---- phase 1 per jb position: scores -> exp -> Z -> w
                        for half in range(4):
                            jb = s * 4 + half
                            wt = wbuf.tile([128, 16, 256], F16, tag="w")
                            zt = None if zdve else zp.tile([128, 256], F32, tag="z")
                            pts = []
                            for g in range(4):
                                sc = scp.tile([128, 4, 256], F32, tag="sc")
                                # slot order puts row-group-0 heads in bank 0
                                # and row-group-64 heads in bank 1: concurrent
                                # different-row-group matmuls must not share a
                                # PSUM bank (HW constraint, not in CoreSim).
                                for hh, h in enumerate(_GRP_HEADS[g]):
                                    c, off = h // 2, (h % 2) * 64
                                    nc.tensor.matmul(
                                        sc[:, hh, :],
                                        kt[off:off + 64, c,
                                           jb * 128:(jb + 1) * 128],
                                        qt[off:off + 64, c, coff:coff + 256],
                                        start=True, stop=True,
                                        tile_position=(off, 0),
                                    )
                                pt = ptp.tile([128, 4, 256], F16, tag="p")
                                nc.scalar.activation(out=pt[:], in_=sc[:],
                                                     func=AF.Exp)
                                pts.append(pt)
                                if stages >= 2 and not zdve:
                                    for hh in range(4):
                                        nc.tensor.matmul(
                                            zt[:], idt[:], pt[:, hh, :],
                                            start=(g == 0 and hh == 0),
                                            stop=(g == 3 and hh == 3),
                                        )
                            if stages >= 3:
                                r32 = rtp.tile([128, 256], F32, tag="r32")
                                if zdve:
                                    t01 = ptp.tile([128, 4, 256], F16, tag="t01")
                                    t23 = ptp.tile([128, 4, 256], F16, tag="t23")
                                    nc.vector.tensor_add(t01[:], pts[0][:], pts[1][:])
                                    nc.vector.tensor_add(t23[:], pts[2][:], pts[3][:])
                                    nc.vector.tensor_add(t01[:], t01[:], t23[:])
                                    u2 = rtp.tile([128, 2, 256], F16, tag="u2")
                                    nc.vector.tensor_add(u2[:], t01[:, 0:2, :], t01[:, 2:4, :])
                                    z32 = rtp.tile([128, 256], F32, tag="z32")
                                    nc.vector.tensor_add(z32[:], u2[:, 0, :], u2[:, 1, :])
                                    nc.vector.reciprocal_approx_fast(out=r32[:], in_=z32[:])
                                else:
                                    nc.vector.reciprocal_approx_fast(out=r32[:], in_=zt[:])
                                mk = mkp.tile([128, 256], F16, tag="mk")
                                nc.sync.dma_start(out=mk[:], in_=masks[poff + jb, :, :])
                                r16 = rtp.tile([128, 256], F16, tag="r16")
                                nc.vector.tensor_mul(r16[:], r32[:], mk[:])
                                rb = r16[:].rearrange("p (a f) -> p a f", a=1) \
                                           .to_broadcast([128, 4, 256])
                                for g in range(4):
                                    nc.vector.tensor_mul(
                                        wt[:, 4 * g:4 * g + 4, :], pts[g][:], rb)
                            wts_s[half] = wt
                        # ---- phase 2: WV matmuls (col-packed head pairs)
                        for pr in range(8 if stages >= 4 else 0):
                            ops_ = otp.tile([128, 256], F32, tag="ot")
                            for sub in range(2):
                                h = 2 * pr + sub
                                po = sub * 64
                                g_, s_ = _HEAD_SLOT[h]
                                for half in range(4):
                                    nc.tensor.matmul(
                                        ops_[po:po + 64, :],
                                        vt[:, s * 4 + half, h * 64:(h + 1) * 64],
                                        wts_s[half][:, 4 * g_ + s_, :],
                                        start=(half == 0), stop=(half == 3),
                                        tile_position=(0, po),
                                    )
                            nc.any.tensor_copy(outp_c[:, s, pr, :], ops_[:])

            # ---- s-reduction + suffix correction (DVE) + output projection
            with tc.tile_pool(name="yps", bufs=2, space="PSUM") as yps, \
                 tc.tile_pool(name="ck", bufs=4) as ckp:
                for ci in range(2 if stages >= 4 else 0):
                    outp_c, nsb = outp_tiles[ci]
                    for pr in range(8):
                        ck = ckp.tile([128, 256], F16, tag="ck")
                        nc.sync.dma_start(out=ck[:], in_=corr[ci, pr, :, :])
                        nc.vector.tensor_add(
                            outp_c[:, 0, pr, :], outp_c[:, 0, pr, :], ck[:])
                        for s in range(1, nsb):
                            nc.vector.tensor_add(
                                outp_c[:, 0, pr, :], outp_c[:, 0, pr, :],
                                outp_c[:, s, pr, :])
                    for ib in range(2 if stages >= 5 else 0):
                        for nt in range(2):
                            acc = yps.tile([128, 512], F32, tag="yacc")
                            for dc in range(8):
                                nc.tensor.matmul(
                                    acc[:],
                                    outp_c[:, 0, dc, ib * 128:(ib + 1) * 128],
                                    wot[:, dc, nt * 512:(nt + 1) * 512],
                                    start=(dc == 0), stop=False,
                                )
                            nc.tensor.matmul(
                                acc[:], ones1[:], bo_sb[:, nt * 512:(nt + 1) * 512],
                                start=False, stop=True,
                            )
                            yt = ysbp.tile([128, 512], F32, tag="yt")
                            nc.vector.tensor_copy(yt[:], acc[:])
                            nc.sync.dma_start(
                                out=y_o[(ci * 2 + ib) * 128:(ci * 2 + ib + 1) * 128,
                                        nt * 512:(nt + 1) * 512],
                                in_=yt[:])
    nc.compile()
    return nc


# ------------------------------------------------------------------- driver
def _masks_for(q):
    """[24, 128, 256] keep-masks for quarter q (padding positions -> 0)."""
    i = np.arange(128)
    tri = (i[:, None] <= i[None, :]).astype(np.float32)   # [j, i], keep j<=i
    ones = np.ones((128, 128), np.float32)
    zeros = np.zeros((128, 128), np.float32)
    out = np.zeros((NPOS, 128, 256), np.float32)
    for ci, (b0, b1) in enumerate(_chunk_blocks(q)):
        cnt_real = b1 + 1                  # real causal j-blocks
        poff = 0 if ci == 0 else CNT[0]
        for p in range(CNT[ci]):
            if p >= cnt_real:
                continue                   # padding: stays zero
            left = tri if p == b0 else (ones if p < b0 else zeros)
            right = tri if p == b1 else (ones if p < b1 else zeros)
            out[poff + p] = np.concatenate([left, right], axis=1)
    return out.astype(np.float16)


def kernel(x, Wq, bq, Wk, bk, Wv, bv, Wo, bo):
    x = np.asarray(x, dtype=np.float32)
    Wq, bq = np.asarray(Wq, np.float32), np.asarray(bq, np.float32)
    Wk, bk = np.asarray(Wk, np.float32), np.asarray(bk, np.float32)
    Wv, bv = np.asarray(Wv, np.float32), np.asarray(bv, np.float32)
    Wo, bo = np.asarray(Wo, np.float32), np.asarray(bo, np.float32)

    if "a" not in _cache:
        _cache["a"] = _build_a()
    if "b" not in _cache:
        _cache["b"] = _build_b()

    def part8(a):  # [1024, N] -> [128, 8, N] partition-major contiguous
        return np.ascontiguousarray(a.reshape(8, 128, -1).transpose(1, 0, 2))

    x_flat = x.reshape(B * T, D)
    wq16, wk16, wv16 = (part8(w.astype(np.float16)) for w in (Wq, Wk, Wv))
    bqT = np.ascontiguousarray(bq.reshape(8, 128).T).astype(np.float32)
    bkT = np.ascontiguousarray((bk / 32.0).reshape(8, 128).T).astype(np.float32)
    bv_row = bv.astype(np.float16)[None, :]
    in_maps_a = []
    for c in range(NC):
        xTs = part8(np.ascontiguousarray(x_flat[c * 512:(c + 1) * 512].T).astype(np.float16))
        in_maps_a.append(dict(xT=xTs, wq=wq16, wk=wk16, wv=wv16,
                              bqT=bqT, bkT=bkT, bv_row=bv_row))
    res_a = run_bass_kernel_spmd(_cache["a"], in_maps_a, core_ids=list(range(NC)))

    def unpart(a):  # [128, C, N] -> [128*C, N]
        return a.transpose(1, 0, 2).reshape(-1, a.shape[2])

    qT_full = [np.concatenate([unpart(res_a.results[b_ * 4 + i]["qT_o"])
                               for i in range(4)], axis=1) for b_ in range(B)]
    kT_full = [np.concatenate([unpart(res_a.results[b_ * 4 + i]["kT_o"])
                               for i in range(4)], axis=1) for b_ in range(B)]
    v_full = [np.concatenate([unpart(res_a.results[b_ * 4 + i]["v_o"])
                              for i in range(4)], axis=0) for b_ in range(B)]

    ident = np.eye(128, dtype=np.float16)
    bo_row = bo.astype(np.float16)[None, :]
    wo16 = part8(Wo.astype(np.float16))
    masks_q = [_masks_for(q) for q in range(4)]

    in_maps_b = []
    for c in range(NC):
        b_, qq = c // 4, c % 4
        (a0, _a1), (b0, _b1) = _chunk_blocks(qq)
        qT_core = np.ascontiguousarray(np.concatenate(
            [qT_full[b_][:, a0 * 128:a0 * 128 + 256],
             qT_full[b_][:, b0 * 128:b0 * 128 + 256]], axis=1))
        vf32 = v_full[b_].astype(np.float32)
        suffix = (vf32.sum(0)[None, :] - np.cumsum(vf32, axis=0)) / 16.0  # [T, D]
        corr = np.zeros((2, 8, 128, 256), np.float32)
        for ci, cblk in enumerate((a0, b0)):
            blk = suffix[cblk * 128: cblk * 128 + 256]          # [256 i, 1024 d]
            corr[ci] = blk.T.reshape(8, 128, 256)
        v16 = np.ascontiguousarray(
            v_full[b_].reshape(16, 128, D).transpose(1, 0, 2))
        in_maps_b.append(dict(
            qT=part8(qT_core), kT=part8(np.ascontiguousarray(kT_full[b_])),
            v=v16, wo=wo16, bo_row=bo_row,
            ident=ident, masks=masks_q[qq], corr=corr.astype(np.float16)))

    res_b = run_bass_kernel_spmd(_cache["b"], in_maps_b, core_ids=list(range(NC)))

    y = np.zeros((B, T, D), np.float32)
    for c in range(NC):
        b_, qq = c // 4, c % 4
        (a0, _), (b0, _) = _chunk_blocks(qq)
        yc = res_b.results[c]["y"]
        y[b_, a0 * 128:a0 * 128 + 256] = yc[:256]
        y[b_, b0 * 128:b0 * 128 + 256] = yc[256:]
    return y

